# revision 45
# baseline (speedup 1.0000x reference)
"""Trainium2 Bass kernel for nn_DecoderModel_54795192762653.

4-layer decoder, B=4, T=1024, D=1024, H=16, K=4 kv-heads, HD=64, F=4096,
V=32000. 8 NeuronCores: pair (2b, 2b+1) handles batch b; within a pair,
core A owns tokens 0..511 and core B owns 512..1023.

v2 (this file): all weights bf16 with host-side pre-tiled contiguous
layouts (2KB+ partition lines), merged N=512 score matmuls with causal
half-skipping (the reference masks in the scrambled l/j coordinates, so
keep iff l_scr >= j_scr; cells with tlt=1,th=0 are fully dropped),
single-pass Wdown streaming into 8 PSUM accumulator banks, pair
AllGather of k+v in one collective kicked before q is computed, the
attention out-projection pair-ReduceScattered in 4 chunks (bf16) so
the collectives overlap the out-proj matmuls, precomputed causal mask
tiles applied on the vector engine, scores+exp staged into 12 live
a-tiles per kv-block followed by a gapless o-accumulation burst (keeps
the PE HAM-warm), and fast approximate reciprocals for the softmax
denominators and layernorm.

Scrambled semantics recap: unit m (m = g*4 + kv) covers q tokens
[64m, 64m+64) viewed as (1024 l x 64 d) with l = 16*t_loc + chblk;
k/v block c = m % 4 covers tokens [256c, 256c+256) viewed as
(1024 j x 64 d) with j = 4*t_loc + h4. Causality is l_scr >= j_scr.
o[l, och] goes to output token l, channel 64m + d.
"""
import sys

sys.path.insert(0, "/opt/trn_rl_repo")

import numpy as np
import ml_dtypes
from contextlib import ExitStack

import concourse.bass as bass
import concourse.tile as tile
from concourse import bacc, mybir
from concourse.bass_utils import run_bass_kernel_spmd
from concourse.masks import make_identity

P = 128
F32 = mybir.dt.float32
F32R = mybir.dt.float32r
BF16 = mybir.dt.bfloat16
U32 = mybir.dt.uint32
AF = mybir.ActivationFunctionType
OP = mybir.AluOpType

D, H, KV, F, L, V, T, B = 1024, 16, 4, 4096, 4, 32000, 1024, 4
HD = D // H
TL = T // 2          # 512 tokens per core
VC = V // 8          # 4000 vocab per core
EPS = 1e-5
PAIRS = [[0, 1], [2, 3], [4, 5], [6, 7]]
ALL8 = [list(range(8))]

_CACHE = {}


def _layer_norm(nc, pools, ps, h_tiles, out_tiles, g_ap, b_ap):
    """Feature-major layernorm over D=1024 (8 partition tiles x 512 tokens).

    out bf16; stats via PE ones-matmuls; rv via fast approx reciprocal."""
    wk, ones_col = pools["wk"], pools["ones_col"]
    s1 = ps.tile([P, 512], F32, name="mm")
    s2 = ps.tile([P, 512], F32, name="mm")
    for r in range(8):
        nc.tensor.matmul(s1[0:1, :], ones_col[:, 0:1], h_tiles[r],
                         start=(r == 0), stop=(r == 7))
    for r in range(8):
        sq = wk.tile([P, 512], F32R, name="ln_sq")
        nc.scalar.activation(sq[:], h_tiles[r], AF.Square)
        nc.tensor.matmul(s2[0:1, :], ones_col[:, 0:1], sq[:],
                         start=(r == 0), stop=(r == 7))
    mu = wk.tile([1, 512], F32, name="ln_mu")
    nc.scalar.mul(mu[:], s1[0:1, :], 1.0 / D)
    e2 = wk.tile([1, 512], F32, name="ln_e2")
    nc.scalar.mul(e2[:], s2[0:1, :], 1.0 / D)
    musq = wk.tile([1, 512], F32, name="ln_musq")
    nc.scalar.activation(musq[:], mu[:], AF.Square)
    var = wk.tile([1, 512], F32, name="ln_var")
    nc.vector.tensor_sub(var[:], e2[:], musq[:])
    sd = wk.tile([1, 512], F32, name="ln_sd")
    nc.scalar.activation(sd[:], var[:], AF.Sqrt, bias=pools["eps"][0:1, :])
    rv = wk.tile([1, 512], F32, name="ln_rv")
    nc.vector.reciprocal_approx_fast(rv[:], sd[:])
    cv = wk.tile([1, 512], F32, name="ln_cv")
    nc.vector.scalar_tensor_tensor(cv[:], mu[:], -1.0, rv[:],
                                   op0=OP.mult, op1=OP.mult)
    rb = wk.tile([P, 512], F32, name="ln_rb")
    nc.gpsimd.partition_broadcast(rb[:], rv[:])
    cb = wk.tile([P, 512], F32, name="ln_cb")
    nc.gpsimd.partition_broadcast(cb[:], cv[:])
    for r in range(8):
        t1 = wk.tile([P, 512], F32, name="ln_t1")
        nc.vector.tensor_mul(t1[:], h_tiles[r], rb[:])
        nc.vector.tensor_add(t1[:], t1[:], cb[:])
        nc.scalar.activation(out_tiles[r], t1[:], AF.Identity,
                             bias=b_ap(r), scale=g_ap(r))


def build_kernel(n_layers=L, dbg=False):
    nc = bacc.Bacc("TRN2", target_bir_lowering=False, debug=False, num_devices=8)
    dbg_d = {}
    if dbg:
        for name, shape, dt in [
                ("d_xh", [P, 8, 512], BF16), ("d_qT", [P, 8, 512], BF16),
                ("d_kTd", [P, 4, T], BF16), ("d_vst", [P, 8, 4, 65], BF16),
                ("d_ost", [4, P, 2, 512], BF16), ("d_hat", [P, 8, 512], F32),
                ("d_h1", [P, 8, 512], F32)]:
            dbg_d[name] = nc.dram_tensor(name, shape, dt, kind="ExternalOutput")

    # ---------------- I/O ----------------
    idx_d = nc.dram_tensor("idx", [TL], U32, kind="ExternalInput")
    pe_d = nc.dram_tensor("pe", [TL, D], F32, kind="ExternalInput")
    emb_d = nc.dram_tensor("emb", [V, D], F32, kind="ExternalInput")
    # pre-tiled bf16 weights (host layouts; see kernel() below)
    wqkv_d = nc.dram_tensor("wqkv", [n_layers, 12, P, 8, P], BF16, kind="ExternalInput")
    wout_d = nc.dram_tensor("wout", [n_layers, 8, P, 4, P], BF16, kind="ExternalInput")
    wup_d = nc.dram_tensor("wup", [n_layers, 16, P, 8, 256], BF16, kind="ExternalInput")
    wgate_d = nc.dram_tensor("wgate", [n_layers, 16, P, 8, 256], BF16, kind="ExternalInput")
    wdown_d = nc.dram_tensor("wdown", [n_layers, 32, P, D], BF16, kind="ExternalInput")
    ln1g_d = nc.dram_tensor("ln1g", [n_layers, D], F32, kind="ExternalInput")
    ln1b_d = nc.dram_tensor("ln1b", [n_layers, D], F32, kind="ExternalInput")
    ln2g_d = nc.dram_tensor("ln2g", [n_layers, D], F32, kind="ExternalInput")
    ln2b_d = nc.dram_tensor("ln2b", [n_layers, D], F32, kind="ExternalInput")
    bup_d = nc.dram_tensor("bup", [n_layers, F], F32, kind="ExternalInput")
    bgate_d = nc.dram_tensor("bgate", [n_layers, F], F32, kind="ExternalInput")
    bdown_d = nc.dram_tensor("bdown", [n_layers, D], F32, kind="ExternalInput")
    flng_d = nc.dram_tensor("flng", [D], F32, kind="ExternalInput")
    flnb_d = nc.dram_tensor("flnb", [D], F32, kind="ExternalInput")
    wlm_d = nc.dram_tensor("wlm", [8, P, VC], BF16, kind="ExternalInput")
    blm_d = nc.dram_tensor("blm", [VC], F32, kind="ExternalInput")
    logits_d = nc.dram_tensor("logits", [B, VC], F32, kind="ExternalOutput")

    # collective bounce buffers (internal DRAM)
    # kv: chunks 0-1 = k (256 ch), 2-3 = v (256 ch), feature-major
    k_ag_in = nc.dram_tensor("k_ag_in", [2, P, TL], BF16)
    k_ag_out = nc.dram_tensor("k_ag_out", [2, 2, P, TL], BF16)
    v_ag_in = nc.dram_tensor("v_ag_in", [2, P, TL], BF16)
    v_ag_out = nc.dram_tensor("v_ag_out", [2, 2, P, TL], BF16)
    # o exchange: out-proj partials, reduce-scattered by token half (4 chunks)
    rs_in = [nc.dram_tensor(f"rs_in{g}", [2, 2, P, TL], BF16) for g in range(4)]
    rs_out = [nc.dram_tensor(f"rs_out{g}", [2, P, TL], BF16) for g in range(4)]
    warm_in = nc.dram_tensor("warm_in", [1, 256], BF16)
    warm_out = nc.dram_tensor("warm_out", [2, 1, 256], BF16)
    fin_ag_in = nc.dram_tensor("fin_ag_in", [D], F32)
    fin_ag_out = nc.dram_tensor("fin_ag_out", [8, D], F32, addr_space="Shared")

    with tile.TileContext(nc) as tc, ExitStack() as ctx:
        pers = ctx.enter_context(tc.tile_pool(name="pers", bufs=1))
        wk = ctx.enter_context(tc.tile_pool(name="wk", bufs=3))
        pools = {"wk": wk}

        # ---------------- constants ----------------
        ones_col = pers.tile([P, 1], F32R, tag="ones_col")
        nc.gpsimd.memset(ones_col[:].bitcast(F32), 1.0)
        pools["ones_col"] = ones_col
        ones_row = pers.tile([1, P], F32R, tag="ones_row")
        nc.gpsimd.memset(ones_row[:].bitcast(F32), 1.0)
        pools["ones_row"] = ones_row
        ident = pers.tile([P, P], F32, tag="ident")
        make_identity(nc, ident[:])
        identb = pers.tile([P, P], BF16, tag="identb")
        nc.vector.tensor_copy(identb[:], ident[:])
        eps_t = pers.tile([P, 1], F32, tag="eps")
        nc.gpsimd.memset(eps_t[:], EPS)
        pools["eps"] = eps_t



        # warm up the collectives path with a tiny pair AllGather so the
        # first real collective doesn't pay the ~11us trigger warmup
        wrm = pers.tile([1, 256], BF16, tag="wrm")
        nc.gpsimd.memset(wrm[:], 0.0)
        nc.sync.dma_start(warm_in[0], wrm[:])
        nc.gpsimd.collective_compute(
            "AllGather", OP.bypass, replica_groups=PAIRS,
            ins=[warm_in[:, :]], outs=[warm_out[:, :, :]])

        # ---------------- per-layer params (small, load all) ----------------
        lnp = {}
        for name, dram, nt in [("ln1g", ln1g_d, 8), ("ln1b", ln1b_d, 8),
                               ("ln2g", ln2g_d, 8), ("ln2b", ln2b_d, 8),
                               ("bup", bup_d, 32), ("bgate", bgate_d, 32),
                               ("bdown", bdown_d, 8)]:
            t = pers.tile([P, n_layers, nt], F32, tag=f"p_{name}")
            nc.sync.dma_start(t[:], dram.ap().rearrange("l (t p) -> p l t", p=P))
            lnp[name] = t
        fln = pers.tile([P, 2, 8], F32, tag="p_fln")
        nc.sync.dma_start(fln[:, 0], flng_d.ap().rearrange("(t p) -> p t", p=P))
        nc.sync.dma_start(fln[:, 1], flnb_d.ap().rearrange("(t p) -> p t", p=P))

        # ---------------- embedding ----------------
        h = pers.tile([P, 8, 512], F32R, tag="h")      # residual stream h^T
        with ExitStack() as ectx:
            ep = ectx.enter_context(tc.tile_pool(name="embed", bufs=4))
            eps_ = ectx.enter_context(tc.tile_pool(name="embps", bufs=2, space="PSUM"))
            idx_t = ep.tile([P, 4], U32, name="idx")
            nc.sync.dma_start(idx_t[:], idx_d.ap().rearrange("(c p) -> p c", p=P))
            for tt in range(4):
                g_nat = ep.tile([P, D], F32, name="g_nat")
                nc.gpsimd.indirect_dma_start(
                    out=g_nat[:], out_offset=None, in_=emb_d[:, :],
                    in_offset=bass.IndirectOffsetOnAxis(ap=idx_t[:, tt:tt + 1],
                                                        axis=0))
                pe_t = ep.tile([P, D], F32, name="pe_t")
                nc.sync.dma_start(pe_t[:], pe_d[tt * P:(tt + 1) * P, :])
                h_nat = ep.tile([P, D], F32, name="h_nat")
                nc.vector.scalar_tensor_tensor(h_nat[:], g_nat[:],
                                               float(np.sqrt(D)), pe_t[:],
                                               op0=OP.mult, op1=OP.add)
                for r in range(8):
                    tr_ps = eps_.tile([P, P], F32, name="mm")
                    nc.tensor.transpose(tr_ps[:, 0:P],
                                        h_nat[:, r * P:(r + 1) * P], ident[:])
                    nc.scalar.copy(h[:, r, tt * P:(tt + 1) * P], tr_ps[:, 0:P])

        # precomputed causal masks for the boundary cells (th == tlt):
        # keep iff 16*tau + 2*hidx + par + 512*th - 4*(tl0+p) - h4 >= 0
        # (one [P, 2, 8, 2, 32] bf16 tile per (tlt, h4); th == tlt).
        # Built after the embedding so the gpsimd work here doesn't block
        # the embedding's indirect-gather DMAs in the gpsimd FIFO.
        cmask = pers.tile([P, 2, 4, 2, 8, 2, 32], BF16, tag="cmask")
        nc.gpsimd.memset(cmask[:], 1.0)
        for tlt in range(2):
            for h4 in range(4):
                nc.gpsimd.affine_select(
                    out=cmask[:, tlt, h4], in_=cmask[:, tlt, h4],
                    pattern=[[1, 2], [2, 8], [0, 2], [16, 32]],
                    channel_multiplier=-4,
                    base=512 * tlt - 4 * (tlt * P) - h4,
                    compare_op=OP.is_ge, fill=0.0)

        # ---------------- layers ----------------
        for ly in range(n_layers):
            # ======== attention ========
            with ExitStack() as lctx:
                ap_ = lctx.enter_context(tc.tile_pool(name=f"attn{ly}", bufs=1))
                apw = lctx.enter_context(tc.tile_pool(name=f"attnw{ly}", bufs=3))
                apa = lctx.enter_context(tc.tile_pool(name=f"attna{ly}", bufs=10))
                ps_s = lctx.enter_context(
                    tc.tile_pool(name=f"pss{ly}", bufs=3, space="PSUM"))
                ps_o = lctx.enter_context(
                    tc.tile_pool(name=f"pso{ly}", bufs=1, space="PSUM"))
                xh = ap_.tile([P, 8, 512], BF16, tag="xh")
                _layer_norm(nc, pools, ps_s,
                            [h[:, r, :] for r in range(8)],
                            [xh[:, r, :] for r in range(8)],
                            lambda r: lnp["ln1g"][:, ly, r:r + 1],
                            lambda r: lnp["ln1b"][:, ly, r:r + 1])

                # ---- qkv: k/v chunks first, then AG, then q ----
                qT = ap_.tile([P, 8, 512], BF16, tag="qT")
                kTl = ap_.tile([P, 2, 512], BF16, tag="kTl")
                vTl = ap_.tile([P, 2, 512], BF16, tag="vTl")
                for ct in [8, 9, 10, 11] + list(range(8)):
                    wc = apw.tile([P, 8, P], BF16, name="wqkv_ct")
                    nc.sync.dma_start(wc[:], wqkv_d[ly, ct])
                    q_ps = ps_s.tile([P, 512], F32, name="mm")
                    for kt in range(8):
                        nc.tensor.matmul(q_ps[:], wc[:, kt, :], xh[:, kt, :],
                                         start=(kt == 0), stop=(kt == 7))
                    if ct < 8:
                        nc.scalar.copy(qT[:, ct, :], q_ps[:])
                    elif ct < 10:
                        nc.scalar.copy(kTl[:, ct - 8, :], q_ps[:])
                    else:
                        nc.scalar.copy(vTl[:, ct - 10, :], q_ps[:])
                    if ct == 9:
                        # kick the pair AllGather of k as soon as it's ready
                        nc.sync.dma_start(
                            k_ag_in.ap().rearrange("ck p t -> p ck t"), kTl[:])
                        nc.gpsimd.collective_compute(
                            "AllGather", OP.bypass, replica_groups=PAIRS,
                            ins=[k_ag_in[:, :, :]], outs=[k_ag_out[:, :, :, :]])
                    if ct == 11:
                        nc.sync.dma_start(
                            v_ag_in.ap().rearrange("ck p t -> p ck t"), vTl[:])
                        nc.gpsimd.collective_compute(
                            "AllGather", OP.bypass, replica_groups=PAIRS,
                            ins=[v_ag_in[:, :, :]], outs=[v_ag_out[:, :, :, :]])

                # ---- stage kTd (dup on both partition halves) ----
                kTd = ap_.tile([P, 4, T], BF16, tag="kTd")
                for half in range(2):
                    src = k_ag_out[half].rearrange(
                        "ck (h2 d) t -> d (ck h2) t", d=64)
                    nc.sync.dma_start(kTd[0:64, :, half * TL:(half + 1) * TL], src)
                    nc.sync.dma_start(kTd[64:128, :, half * TL:(half + 1) * TL], src)
                # ---- stage v token-major via PE transposes ----
                vT = ap_.tile([P, 2, T], BF16, tag="vT")
                for half in range(2):
                    nc.sync.dma_start(
                        vT[:, :, half * TL:(half + 1) * TL],
                        v_ag_out[half].rearrange("ck p t -> p ck t"))
                vst = ap_.tile([P, 8, 4, 65], BF16, tag="vst")
                nc.gpsimd.memset(vst[:, :, :, 64:65], 1.0)
                for tt8 in range(8):
                    for ck in range(2):
                        vt_ps = ps_s.tile([P, P], BF16, name="mm")
                        nc.tensor.transpose(
                            vt_ps[:], vT[:, ck, tt8 * P:(tt8 + 1) * P], identb[:])
                        nc.scalar.copy(
                            vst[:, tt8, 2 * ck:2 * ck + 2, 0:64],
                            vt_ps[:].rearrange("p (h2 d) -> p h2 d", d=64))

                # ---- attention: 4 kv blocks, causal half-skip ----
                # cells (tlt, h4, th); skip (tlt=1, th=0); mask iff th == tlt
                ost = [ap_.tile([P, 2, 512], BF16, tag=f"ost{r}", name=f"ost{r}")
                       for r in range(4)]
                qTr = qT[:].rearrange("p h (blk tau) -> p h blk tau", tau=64)
                for c in range(4):
                    for th in range(2):
                        seq = [(tlt, h4) for tlt in range(2) for h4 in range(4)
                               if not (tlt == 1 and th == 0)]
                        o_ps = [ps_o.tile([P, 512], F32, name=f"oacc{u}")
                                for u in range(2)]
                        # phase A: score matmuls + exp + mask into a_tiles
                        a_tiles = {}
                        for tlt, h4 in seq:
                            tl0 = tlt * P
                            a_tile = apa.tile([P, 2, 8, 2, 32], BF16,
                                              name="a_tile")
                            a_tiles[(tlt, h4)] = a_tile
                            s_ps = ps_s.tile([P, 2, 512], F32, name="mm")
                            for par in range(2):
                                b0 = par * 64
                                lhsT = kTd[b0:b0 + 64, h4,
                                           c * 256 + tl0:c * 256 + tl0 + P]
                                rhs = qTr[b0:b0 + 64, :, c::4,
                                          th * 32:(th + 1) * 32]
                                nc.tensor.matmul(s_ps[:, par, :], lhsT, rhs,
                                                 start=True, stop=True)
                            nc.scalar.activation(
                                a_tile[:].rearrange("p q h u t -> p (q h u t)"),
                                s_ps[:].rearrange("p q t -> p (q t)"),
                                AF.Exp, scale=0.125)
                            if th == tlt:
                                nc.vector.tensor_mul(
                                    a_tile[:].rearrange(
                                        "p q h u t -> p (q h u t)"),
                                    a_tile[:].rearrange(
                                        "p q h u t -> p (q h u t)"),
                                    cmask[:, tlt, h4].rearrange(
                                        "p q h u t -> p (q h u t)"))
                        # phase B: o-accumulation matmuls, back to back
                        for i, (tlt, h4) in enumerate(seq):
                            tt8 = c * 2 + tlt
                            for u in range(2):
                                nc.tensor.matmul(
                                    o_ps[u][0:65, :],
                                    vst[:, tt8, h4, :],
                                    a_tiles[(tlt, h4)][:, :, :, u, :],
                                    start=(i == 0), stop=(i == len(seq) - 1))
                        # normalize + store to ost for this th
                        for u in range(2):
                            r = u * 2 + (c // 2)
                            den = wk.tile([1, 512], F32, name="den")
                            nc.scalar.copy(den[:], o_ps[u][64:65, :])
                            rcp = wk.tile([1, 512], F32, name="rcp")
                            nc.vector.reciprocal_approx_fast(rcp[:], den[:])
                            rcb = wk.tile([64, 512], F32, name="rcb")
                            nc.gpsimd.partition_broadcast(rcb[:], rcp[:])
                            nc.vector.tensor_mul(
                                ost[r][(c % 2) * 64:(c % 2) * 64 + 64, th, :],
                                o_ps[u][0:64, :], rcb[:])
                if dbg and ly == 0:
                    nc.sync.dma_start(dbg_d["d_xh"].ap(), xh[:])
                    nc.sync.dma_start(dbg_d["d_qT"].ap(), qT[:])
                    nc.sync.dma_start(dbg_d["d_kTd"].ap(), kTd[:])
                    nc.sync.dma_start(dbg_d["d_vst"].ap(), vst[:])
                    for r in range(4):
                        nc.sync.dma_start(dbg_d["d_ost"][r], ost[r][:])
                # ---- out-proj partials (local och half, both l halves),
                #      then pair ReduceScatter by token half, split in two
                #      so RS#1 overlaps the rout 4-7 matmuls ----
                for grp in range(4):
                    for ri in range(2):
                        rout = grp * 2 + ri
                        woc = apw.tile([P, 4, P], BF16, name="wocol")
                        nc.sync.dma_start(woc[:], wout_d[ly, rout])
                        for lh in range(2):
                            p_ps = ps_s.tile([P, 512], F32, name="mm")
                            for kt in range(4):
                                rhs = ost[kt][:, lh, :].rearrange(
                                    "p (par hh tau) -> p tau hh par",
                                    par=2, hh=8)
                                nc.tensor.matmul(p_ps[:], woc[:, kt, :], rhs,
                                                 start=(kt == 0), stop=(kt == 3))
                            ap_sb = wk.tile([P, 512], BF16, name="ap_sb")
                            nc.vector.tensor_copy(ap_sb[:], p_ps[:])
                            nc.sync.dma_start(rs_in[grp][lh, ri], ap_sb[:])
                    nc.gpsimd.collective_compute(
                        "ReduceScatter", OP.add, replica_groups=PAIRS,
                        ins=[rs_in[grp][:, :, :, :]],
                        outs=[rs_out[grp][:, :, :]])
                for grp in range(4):
                    at = ap_.tile([P, 2, 512], BF16, tag=f"at_sb{grp}")
                    nc.sync.dma_start(at[:], rs_out[grp].ap().rearrange(
                        "r p t -> p r t"))
                    for ri in range(2):
                        rout = grp * 2 + ri
                        nc.vector.tensor_add(h[:, rout, :], h[:, rout, :],
                                             at[:, ri, :])
                if dbg and ly == 0:
                    nc.sync.dma_start(dbg_d["d_hat"].ap(),
                                      h[:].bitcast(F32))

            # ======== FFN ========
            with ExitStack() as fctx:
                fp = fctx.enter_context(tc.tile_pool(name=f"ffn{ly}", bufs=1))
                fpw = fctx.enter_context(tc.tile_pool(name=f"ffnw{ly}", bufs=3))
                fps = fctx.enter_context(tc.tile_pool(name=f"ffns{ly}", bufs=2))
                x2 = fp.tile([P, 8, 512], BF16, tag="x2")
                with ExitStack() as ugctx:
                    ps_ug = ugctx.enter_context(
                        tc.tile_pool(name=f"psug{ly}", bufs=4, space="PSUM"))
                    _layer_norm(nc, pools, ps_ug,
                                [h[:, r, :] for r in range(8)],
                                [x2[:, r, :] for r in range(8)],
                                lambda r: lnp["ln2g"][:, ly, r:r + 1],
                                lambda r: lnp["ln2b"][:, ly, r:r + 1])
                    hg = fp.tile([P, 32, 512], BF16, tag="hg")
                    for ch in range(16):          # F chunks of 256
                        wu = fpw.tile([P, 8, 256], BF16, name="wup")
                        nc.sync.dma_start(wu[:], wup_d[ly, ch])
                        wg = fpw.tile([P, 8, 256], BF16, name="wgate")
                        nc.sync.dma_start(wg[:], wgate_d[ly, ch])
                        for fi in range(2):       # F-tiles of 128 in chunk
                            ft = ch * 2 + fi
                            u_ps = ps_ug.tile([P, 512], F32, name="mm")
                            for kt in range(8):
                                nc.tensor.matmul(
                                    u_ps[:], wu[:, kt, fi * P:(fi + 1) * P],
                                    x2[:, kt, :],
                                    start=(kt == 0), stop=(kt == 7))
                            g_ps = ps_ug.tile([P, 512], F32, name="mm")
                            for kt in range(8):
                                nc.tensor.matmul(
                                    g_ps[:], wg[:, kt, fi * P:(fi + 1) * P],
                                    x2[:, kt, :],
                                    start=(kt == 0), stop=(kt == 7))
                            u_sb = fps.tile([P, 512], BF16, name="u_sb")
                            nc.scalar.activation(
                                u_sb[:], u_ps[:], AF.Identity,
                                bias=lnp["bup"][:, ly, ft:ft + 1])
                            g_sb = fps.tile([P, 512], BF16, name="g_sb")
                            nc.scalar.activation(
                                g_sb[:], g_ps[:], AF.Gelu_apprx_tanh,
                                bias=lnp["bgate"][:, ly, ft:ft + 1])
                            nc.vector.tensor_mul(hg[:, ft, :], u_sb[:], g_sb[:])
                # down: single pass over Wdown, 8 PSUM accumulator banks
                with ExitStack() as dctx:
                    ps_d = dctx.enter_context(
                        tc.tile_pool(name=f"psd{ly}", bufs=1, space="PSUM"))
                    d_ps = [ps_d.tile([P, 512], F32, name=f"dacc{i}")
                            for i in range(8)]
                    for kt in range(32):
                        wd = fpw.tile([P, D], BF16, name="wdown")
                        nc.sync.dma_start(wd[:], wdown_d[ly, kt])
                        for i in range(8):
                            nc.tensor.matmul(d_ps[i][:],
                                             wd[:, i * P:(i + 1) * P],
                                             hg[:, kt, :],
                                             start=(kt == 0), stop=(kt == 31))
                    for i in range(8):
                        dn = fps.tile([P, 512], F32, name="dn_sb")
                        nc.scalar.activation(dn[:], d_ps[i][:], AF.Identity,
                                             bias=lnp["bdown"][:, ly, i:i + 1])
                        nc.vector.tensor_add(h[:, i, :], h[:, i, :], dn[:])
            if dbg and ly == 0:
                nc.sync.dma_start(dbg_d["d_h1"].ap(), h[:].bitcast(F32))

        # -------- final LN (local last token, col 511) + AG + LM head --------
        with ExitStack() as tctx:
            tp = tctx.enter_context(tc.tile_pool(name="tail", bufs=2))
            tps = tctx.enter_context(tc.tile_pool(name="tailps", bufs=3,
                                                  space="PSUM"))
            s1 = tps.tile([P, 512], F32, name="mm")
            s2 = tps.tile([P, 512], F32, name="mm")
            # fp32r matmuls need an even moving free dim: do 2 cols, use col 1
            for r in range(8):
                nc.tensor.matmul(s1[0:1, 0:2], ones_col[:, 0:1], h[:, r, 510:512],
                                 start=(r == 0), stop=(r == 7))
            for r in range(8):
                sqf = tp.tile([P, 2], F32R, name="fln_sq")
                nc.scalar.activation(sqf[:], h[:, r, 510:512], AF.Square)
                nc.tensor.matmul(s2[0:1, 0:2], ones_col[:, 0:1], sqf[:],
                                 start=(r == 0), stop=(r == 7))
            muf = tp.tile([1, 1], F32, name="fln_mu")
            nc.scalar.mul(muf[:], s1[0:1, 1:2], 1.0 / D)
            e2f = tp.tile([1, 1], F32, name="fln_e2")
            nc.scalar.mul(e2f[:], s2[0:1, 1:2], 1.0 / D)
            musqf = tp.tile([1, 1], F32, name="fln_musq")
            nc.scalar.activation(musqf[:], muf[:], AF.Square)
            varf = tp.tile([1, 1], F32, name="fln_var")
            nc.vector.tensor_sub(varf[:], e2f[:], musqf[:])
            sdf = tp.tile([1, 1], F32, name="fln_sd")
            nc.scalar.activation(sdf[:], varf[:], AF.Sqrt, bias=eps_t[0:1, :])
            rvf = tp.tile([1, 1], F32, name="fln_rv")
            nc.vector.reciprocal(rvf[:], sdf[:])
            cvf = tp.tile([1, 1], F32, name="fln_cv")
            nc.vector.scalar_tensor_tensor(cvf[:], muf[:], -1.0, rvf[:],
                                           op0=OP.mult, op1=OP.mult)
            rbf = tp.tile([P, 1], F32, name="fln_rb")
            nc.gpsimd.partition_broadcast(rbf[:], rvf[:])
            cbf = tp.tile([P, 1], F32, name="fln_cb")
            nc.gpsimd.partition_broadcast(cbf[:], cvf[:])
            hfin = tp.tile([P, 8], F32, name="hfin")
            for r in range(8):
                t2 = tp.tile([P, 1], F32, name="fln_t2")
                nc.vector.tensor_mul(t2[:], h[:, r, 511:512], rbf[:])
                nc.vector.tensor_add(t2[:], t2[:], cbf[:])
                nc.scalar.activation(hfin[:, r:r + 1], t2[:], AF.Identity,
                                     bias=fln[:, 1, r:r + 1],
                                     scale=fln[:, 0, r:r + 1])
            nc.sync.dma_start(fin_ag_in.ap().rearrange("(r p) -> p r", p=P),
                              hfin[:])
            nc.gpsimd.collective_compute(
                "AllGather", OP.bypass, replica_groups=ALL8,
                ins=[fin_ag_in[:]], outs=[fin_ag_out[:, :]])

            # LM head: my vocab chunk (VC=4000) for all 4 batches
            hallf = tp.tile([P, 8, 4], F32, name="hallf")
            for bb in range(4):
                nc.sync.dma_start(
                    hallf[:, :, bb],
                    fin_ag_out[2 * bb + 1].rearrange("(r p) -> p r", p=P))
            hall = tp.tile([P, 8, 4], BF16, name="hall")
            nc.vector.tensor_copy(hall[:].rearrange("p k b -> p (k b)"),
                                  hallf[:].rearrange("p k b -> p (k b)"))
            tpw = tctx.enter_context(tc.tile_pool(name="tailw", bufs=8))
            for nt in range(8):
                n0, n1 = nt * 500, (nt + 1) * 500
                l_ps = tps.tile([P, 512], F32, name="mm")
                for kt in range(8):
                    wl = tpw.tile([P, 500], BF16, name="wlm")
                    nc.sync.dma_start(wl[:], wlm_d[kt, :, n0:n1])
                    nc.tensor.matmul(l_ps[0:4, 0:500], hall[:, kt, :], wl[:],
                                     start=(kt == 0), stop=(kt == 7))
                bl = tp.tile([1, 500], F32, name="blm")
                nc.sync.dma_start(bl[:], blm_d[n0:n1])
                blb = tp.tile([4, 500], F32, name="blb")
                nc.gpsimd.partition_broadcast(blb[:], bl[:])
                lo = tp.tile([4, 512], F32, name="lo")
                nc.vector.tensor_add(lo[:, 0:500], l_ps[0:4, 0:500], blb[:])
                nc.sync.dma_start(logits_d[:, n0:n1], lo[:, 0:500])

    nc.compile()
    return nc


def _pe_table(t, d):
    pos = np.arange(t, dtype=np.float32)[:, None]
    freq = np.exp(-(np.arange(0, d, 2, dtype=np.float32) / d) * np.log(10000.0))
    ang = pos * freq[None, :]
    pe = np.zeros((t, d), dtype=np.float32)
    pe[:, 0::2] = np.sin(ang)
    pe[:, 1::2] = np.cos(ang)
    return pe


def kernel(idx, emb, Wqkv, Wout, ln1_g, ln1_b, ln2_g, ln2_b, Wup, bup,
           Wgate, bgate, Wdown, bdown, fln_g, fln_b, Wlm, blm, _trace=False,
           _dbg=False):
    f32 = lambda x: np.ascontiguousarray(np.asarray(x, dtype=np.float32))
    bf16 = lambda x: np.ascontiguousarray(
        np.asarray(x, dtype=np.float32).astype(ml_dtypes.bfloat16))
    idx = np.asarray(idx)
    nl = int(np.asarray(Wqkv).shape[0])

    # host-side pre-tiling into partition-contiguous bf16 layouts
    wqkv_t = bf16(np.asarray(Wqkv, dtype=np.float32)
                  .reshape(nl, 8, P, 12, P).transpose(0, 3, 2, 1, 4))
    wout_f = np.asarray(Wout, dtype=np.float32)
    wup_t = bf16(np.asarray(Wup, dtype=np.float32)
                 .reshape(nl, 8, P, 16, 256).transpose(0, 3, 2, 1, 4))
    wgate_t = bf16(np.asarray(Wgate, dtype=np.float32)
                   .reshape(nl, 8, P, 16, 256).transpose(0, 3, 2, 1, 4))
    wdown_t = bf16(np.asarray(Wdown, dtype=np.float32).reshape(nl, 32, P, D))
    wlm_t = None  # per-core below
    emb_f = f32(emb)

    if ("nc", nl, _dbg) not in _CACHE:
        _CACHE[("nc", nl, _dbg)] = build_kernel(nl, dbg=_dbg)
    nc = _CACHE[("nc", nl, _dbg)]

    pe = _pe_table(T, D)
    wlm_full = np.asarray(Wlm, dtype=np.float32)
    blm_f = f32(blm)
    in_maps = []
    for core in range(8):
        b, half = core // 2, core % 2
        t0 = half * TL
        wlm_t = bf16(wlm_full[:, core * VC:(core + 1) * VC].reshape(8, P, VC))
        # out-proj weights: this core's o-channel rows (= its token half)
        wout_t = bf16(wout_f[:, t0:t0 + TL, :]
                      .reshape(nl, 4, P, 8, P).transpose(0, 3, 2, 1, 4))
        in_maps.append({
            "idx": np.ascontiguousarray(idx[b, t0:t0 + TL]).astype(np.uint32),
            "pe": np.ascontiguousarray(pe[t0:t0 + TL]),
            "emb": emb_f,
            "wqkv": wqkv_t,
            "wout": wout_t,
            "wup": wup_t, "wgate": wgate_t, "wdown": wdown_t,
            "ln1g": f32(ln1_g), "ln1b": f32(ln1_b),
            "ln2g": f32(ln2_g), "ln2b": f32(ln2_b),
            "bup": f32(bup), "bgate": f32(bgate), "bdown": f32(bdown),
            "flng": f32(fln_g), "flnb": f32(fln_b),
            "wlm": wlm_t,
            "blm": np.ascontiguousarray(blm_f[core * VC:(core + 1) * VC]),
        })
    res = run_bass_kernel_spmd(nc, in_maps, core_ids=list(range(8)),
                               trace=_trace)
    if _dbg:
        return res.results
    logits = np.zeros((B, 1, V), dtype=np.float32)
    for core in range(8):
        logits[:, 0, core * VC:(core + 1) * VC] = res.results[core]["logits"]
    if _trace:
        return logits, res
    return logits
